# revision 8
# baseline (speedup 1.0000x reference)
"""Trainium2 Bass kernel for a pre-RMSNorm attention+FFN transformer block.

Problem: x (2, 1024, 4096) fp32, channel-major (B, C, T).
  h = x^T; h += Attn(RMSNorm(h)); h += FFN(RMSNorm(h)); return h^T.

Sharding: 8 cores = 2 batches x 4 query-token chunks of 1024.  Each core
computes K/V for its batch's full 4096 tokens (redundantly within the
4-core group -- avoids all collectives), attention + Wo + FFN for its own
1024-token chunk.  Host slices inputs and concatenates the 8 output chunks.

All matmuls run in bf16 (fp32 PSUM accumulation); the residual path stays
fp32.  Everything is kept channel-major so no transposes are needed:
x[b] is already (C, T), weights load in natural layout.
"""

import numpy as np
import ml_dtypes

import concourse.bass as bass
import concourse.mybir as mybir
import concourse.tile as tile
from concourse import bacc
from concourse.bass_utils import run_bass_kernel_spmd

F32 = mybir.dt.float32
BF16 = mybir.dt.bfloat16
AF = mybir.ActivationFunctionType

B = 2
C = 1024
T = 4096
TQ = 1024          # query-token chunk per core
H = 4
DH = 256
FF = 1536
P = 128
NT = 512           # moving-operand / PSUM tile width
CT = C // P        # 8 channel tiles
TT = T // NT       # 8 full-T token tiles
TQT = TQ // NT     # 2 chunk token tiles
DB = C // P        # 8 output-channel blocks for q/k/v/o
FFB = FF // P      # 12 ff blocks
TJ = T // P        # 32 key-token blocks

_CACHE = {}


def _rmsnorm_tiles(nc, pool, psum_pool, ones_t, eps_t, xt, aT, tags):
    """xt: [P, CT*NT] fp32 (channel-major, CT c-tiles of one NT-token tile).
    Writes aT [P, CT*NT] bf16 = xt * rsqrt(mean_c(xt^2) + eps)."""
    sqtag, sstag, sqttag, rntag = tags
    ss = psum_pool.tile([P, NT], F32, tag=sstag, name="ss_" + sstag)
    for ci in range(CT):
        sq = pool.tile([P, NT], BF16, tag=sqtag, bufs=2, name="sq_" + sqtag)
        nc.scalar.activation(sq[:], xt[:, ci * NT:(ci + 1) * NT], AF.Square)
        nc.tensor.matmul(ss[:], ones_t[:], sq[:], start=(ci == 0), stop=(ci == CT - 1))
    sqt = pool.tile([P, NT], F32, tag=sqttag, bufs=2, name="sqt_" + sqttag)
    nc.scalar.activation(sqt[:], ss[:], AF.Sqrt, scale=1.0 / C, bias=eps_t[:])
    rn = pool.tile([P, NT], F32, tag=rntag, bufs=2, name="rn_" + rntag)
    nc.vector.reciprocal(rn[:], sqt[:])
    for ci in range(CT):
        nc.vector.tensor_mul(aT[:, ci * NT:(ci + 1) * NT],
                             xt[:, ci * NT:(ci + 1) * NT], rn[:])


def _build():
    nc = bacc.Bacc()
    xb = nc.dram_tensor("xb", [C, T], F32, kind="ExternalInput")
    xq = nc.dram_tensor("xq", [C, TQ], F32, kind="ExternalInput")
    wq = nc.dram_tensor("wq", [C, C], BF16, kind="ExternalInput")
    wk = nc.dram_tensor("wk", [C, C], BF16, kind="ExternalInput")
    wv = nc.dram_tensor("wv", [C, C], BF16, kind="ExternalInput")
    wo = nc.dram_tensor("wo", [C, C], BF16, kind="ExternalInput")
    w1 = nc.dram_tensor("w1", [C, FF], BF16, kind="ExternalInput")
    w2 = nc.dram_tensor("w2", [FF, C], BF16, kind="ExternalInput")
    out = nc.dram_tensor("out", [C, TQ], F32, kind="ExternalOutput")

    with tile.TileContext(nc) as tc:
        cpool_cm = tc.tile_pool(name="const", bufs=1)
        cpool = cpool_cm.__enter__()
        ones_t = cpool.tile([P, P], BF16, tag="ones", name="ones_t")
        nc.vector.memset(ones_t[:], 1.0)
        eps_t = cpool.tile([P, 1], F32, tag="eps", name="eps_t")
        nc.vector.memset(eps_t[:], 1e-8)

        kv_cm = tc.tile_pool(name="kvpool", bufs=1)
        kvp = kv_cm.__enter__()
        kT = kvp.tile([P, DB * T], BF16, tag="kT", name="kT")          # 64KB/part
        vB = kvp.tile([P, TJ * C], BF16, tag="vB", name="vB")          # 64KB/part

        # ---------------- phase A: full-T rmsnorm + K, V ----------------
        pa_cm = tc.tile_pool(name="pa", bufs=1)
        pap = pa_cm.__enter__()
        paps_cm = tc.tile_pool(name="pa_ps", bufs=2, space="PSUM")
        paps = paps_cm.__enter__()
        wk_sb = pap.tile([P, CT * C], BF16, tag="wk_sb", name="wk_sb")
        wv_sb = pap.tile([P, CT * C], BF16, tag="wv_sb", name="wv_sb")
        for ci in range(CT):
            nc.sync.dma_start(wk_sb[:, ci * C:(ci + 1) * C], wk[ci * P:(ci + 1) * P, :])
            nc.sync.dma_start(wv_sb[:, ci * C:(ci + 1) * C], wv[ci * P:(ci + 1) * P, :])
        for tt in range(TT):
            xt = pap.tile([P, CT * NT], F32, tag="xa", bufs=1, name="xa")
            for ci in range(CT):
                nc.sync.dma_start(xt[:, ci * NT:(ci + 1) * NT],
                                  xb[ci * P:(ci + 1) * P, tt * NT:(tt + 1) * NT])
            aT = pap.tile([P, CT * NT], BF16, tag="aT", bufs=1, name="aT")
            _rmsnorm_tiles(nc, pap, paps, ones_t, eps_t, xt, aT, ("sqa", "ssa", "sqta", "rna"))
            for db in range(DB):
                pk = paps.tile([P, NT], F32, tag="pk", name="pk")
                for ci in range(CT):
                    nc.tensor.matmul(pk[:],
                                     wk_sb[:, ci * C + db * P: ci * C + (db + 1) * P],
                                     aT[:, ci * NT:(ci + 1) * NT],
                                     start=(ci == 0), stop=(ci == CT - 1))
                nc.vector.tensor_copy(kT[:, db * T + tt * NT: db * T + (tt + 1) * NT], pk[:])
            for tb in range(NT // P):
                j = tt * (NT // P) + tb
                for hf in range(2):
                    pv = paps.tile([P, NT], F32, tag="pv", name="pv")
                    for ci in range(CT):
                        nc.tensor.matmul(pv[:],
                                         aT[:, ci * NT + tb * P: ci * NT + (tb + 1) * P],
                                         wv_sb[:, ci * C + hf * NT: ci * C + (hf + 1) * NT],
                                         start=(ci == 0), stop=(ci == CT - 1))
                    nc.vector.tensor_copy(vB[:, j * C + hf * NT: j * C + (hf + 1) * NT], pv[:])
        paps_cm.__exit__(None, None, None)
        pa_cm.__exit__(None, None, None)

        # ---------------- phase B: chunk rmsnorm + Q ----------------
        qo_cm = tc.tile_pool(name="qopool", bufs=1, side="right")
        qop = qo_cm.__enter__()
        qT = qop.tile([P, DB * TQ], BF16, tag="qT", name="qT")          # 16KB
        pb_cm = tc.tile_pool(name="pb", bufs=1)
        pbp = pb_cm.__enter__()
        pbps_cm = tc.tile_pool(name="pb_ps", bufs=2, space="PSUM")
        pbps = pbps_cm.__enter__()
        wq_sb = pbp.tile([P, CT * C], BF16, tag="wq_sb", name="wq_sb")
        for ci in range(CT):
            nc.sync.dma_start(wq_sb[:, ci * C:(ci + 1) * C], wq[ci * P:(ci + 1) * P, :])
        for t2 in range(TQT):
            ss = pbps.tile([P, NT], F32, tag="ssb", name="ssb")
            for ci in range(CT):
                xt1 = pbp.tile([P, NT], F32, tag="xb1", bufs=2, name="xb1")
                nc.sync.dma_start(xt1[:], xq[ci * P:(ci + 1) * P, t2 * NT:(t2 + 1) * NT])
                sq = pbp.tile([P, NT], BF16, tag="sqb", bufs=2, name="sqb")
                nc.scalar.activation(sq[:], xt1[:], AF.Square)
                nc.tensor.matmul(ss[:], ones_t[:], sq[:], start=(ci == 0), stop=(ci == CT - 1))
            sqt = pbp.tile([P, NT], F32, tag="sqtb", bufs=2, name="sqtb")
            nc.scalar.activation(sqt[:], ss[:], AF.Sqrt, scale=1.0 / C, bias=eps_t[:])
            rn = pbp.tile([P, NT], F32, tag="rnb", bufs=2, name="rnb")
            nc.vector.reciprocal(rn[:], sqt[:])
            aT = pbp.tile([P, CT * NT], BF16, tag="aTb", bufs=1, name="aTb")
            for ci in range(CT):
                xt2 = pbp.tile([P, NT], F32, tag="xb2", bufs=2, name="xb2")
                nc.sync.dma_start(xt2[:], xq[ci * P:(ci + 1) * P, t2 * NT:(t2 + 1) * NT])
                nc.vector.tensor_mul(aT[:, ci * NT:(ci + 1) * NT], xt2[:], rn[:])
            for db in range(DB):
                pq = pbps.tile([P, NT], F32, tag="pq", name="pq")
                for ci in range(CT):
                    nc.tensor.matmul(pq[:],
                                     wq_sb[:, ci * C + db * P: ci * C + (db + 1) * P],
                                     aT[:, ci * NT:(ci + 1) * NT],
                                     start=(ci == 0), stop=(ci == CT - 1))
                nc.vector.tensor_copy(qT[:, db * TQ + t2 * NT: db * TQ + (t2 + 1) * NT], pq[:])
        pbps_cm.__exit__(None, None, None)
        pb_cm.__exit__(None, None, None)

        # ---------------- phase C: attention ----------------
        oT = qop.tile([P, DB * TQ], BF16, tag="oT", name="oT")          # 16KB
        pc_cm = tc.tile_pool(name="pc", bufs=1)
        pcp = pc_cm.__enter__()
        pss_cm = tc.tile_pool(name="ps_s", bufs=2, space="PSUM")
        pss = pss_cm.__enter__()
        pso_cm = tc.tile_pool(name="ps_o", bufs=2, space="PSUM")
        pso = pso_cm.__enter__()
        NHALF = 16
        for h in range(H):
            for ti in range(TQT):
                po0 = pso.tile([P, NT], F32, tag="po0", name="po0")
                po1 = pso.tile([P, NT], F32, tag="po1", name="po1")
                pr = pso.tile([P, NT], F32, tag="pr", name="pr")
                for half in range(2):
                    et = pcp.tile([P, NHALF * NT], BF16, tag="exp", bufs=2, name="et")
                    for jj in range(NHALF):
                        tj = half * NHALF + jj
                        psc = pss.tile([P, NT], F32, tag="s", name="psc")
                        for dd in range(2):
                            db = 2 * h + dd
                            nc.tensor.matmul(psc[:],
                                             kT[:, db * T + tj * P: db * T + (tj + 1) * P],
                                             qT[:, db * TQ + ti * NT: db * TQ + (ti + 1) * NT],
                                             start=(dd == 0), stop=(dd == 1))
                        nc.scalar.activation(et[:, jj * NT:(jj + 1) * NT], psc[:],
                                             AF.Exp, scale=float(DH) ** -0.5)
                    for jj in range(NHALF):
                        tj = half * NHALF + jj
                        st, sp = (tj == 0), (tj == TJ - 1)
                        e_sl = et[:, jj * NT:(jj + 1) * NT]
                        nc.tensor.matmul(po0[:], vB[:, tj * C + h * DH: tj * C + h * DH + P],
                                         e_sl, start=st, stop=sp, skip_group_check=True)
                        nc.tensor.matmul(po1[:], vB[:, tj * C + h * DH + P: tj * C + (h + 1) * DH],
                                         e_sl, start=st, stop=sp, skip_group_check=True)
                        nc.tensor.matmul(pr[:], ones_t[:], e_sl,
                                         start=st, stop=sp, skip_group_check=True)
                rec = pcp.tile([P, NT], F32, tag="rec", bufs=2, name="rec")
                nc.vector.reciprocal(rec[:], pr[:])
                nc.vector.tensor_mul(oT[:, (2 * h) * TQ + ti * NT:(2 * h) * TQ + (ti + 1) * NT],
                                     po0[:], rec[:])
                nc.vector.tensor_mul(oT[:, (2 * h + 1) * TQ + ti * NT:(2 * h + 1) * TQ + (ti + 1) * NT],
                                     po1[:], rec[:])
        pso_cm.__exit__(None, None, None)
        pss_cm.__exit__(None, None, None)
        pc_cm.__exit__(None, None, None)
        kv_cm.__exit__(None, None, None)

        # ---------------- phase D: Wo + residual ----------------
        h_cm = tc.tile_pool(name="hpool", bufs=1)
        hp = h_cm.__enter__()
        hB = hp.tile([P, CT * TQ], F32, tag="hB", name="hB")            # 32KB
        pd_cm = tc.tile_pool(name="pd", bufs=1)
        pdp = pd_cm.__enter__()
        pdps_cm = tc.tile_pool(name="pd_ps", bufs=2, space="PSUM")
        pdps = pdps_cm.__enter__()
        wo_sb = pdp.tile([P, CT * C], BF16, tag="wo_sb", name="wo_sb")
        for ci in range(CT):
            nc.sync.dma_start(wo_sb[:, ci * C:(ci + 1) * C], wo[ci * P:(ci + 1) * P, :])
        xqD = pdp.tile([P, CT * TQ], F32, tag="xqD", name="xqD")        # 32KB
        for ci in range(CT):
            nc.sync.dma_start(xqD[:, ci * TQ:(ci + 1) * TQ], xq[ci * P:(ci + 1) * P, :])
        for cb in range(CT):
            for t2 in range(TQT):
                ph = pdps.tile([P, NT], F32, tag="ph", name="ph")
                for cp_ in range(CT):
                    nc.tensor.matmul(ph[:],
                                     wo_sb[:, cp_ * C + cb * P: cp_ * C + (cb + 1) * P],
                                     oT[:, cp_ * TQ + t2 * NT: cp_ * TQ + (t2 + 1) * NT],
                                     start=(cp_ == 0), stop=(cp_ == CT - 1))
                nc.vector.tensor_add(hB[:, cb * TQ + t2 * NT: cb * TQ + (t2 + 1) * NT],
                                     ph[:], xqD[:, cb * TQ + t2 * NT: cb * TQ + (t2 + 1) * NT])
        pdps_cm.__exit__(None, None, None)
        pd_cm.__exit__(None, None, None)
        qo_cm.__exit__(None, None, None)

        # ---------------- phase E: FFN ----------------
        pe_cm = tc.tile_pool(name="pe", bufs=1)
        pep = pe_cm.__enter__()
        peps_cm = tc.tile_pool(name="pe_ps", bufs=2, space="PSUM")
        peps = peps_cm.__enter__()
        w1_sb = pep.tile([P, CT * FF], BF16, tag="w1_sb", name="w1_sb")   # 24KB
        for ci in range(CT):
            nc.sync.dma_start(w1_sb[:, ci * FF:(ci + 1) * FF], w1[ci * P:(ci + 1) * P, :])
        w2_sb = pep.tile([P, FFB * C], BF16, tag="w2_sb", name="w2_sb")   # 24KB
        for fi in range(FFB):
            nc.sync.dma_start(w2_sb[:, fi * C:(fi + 1) * C], w2[fi * P:(fi + 1) * P, :])
        fB = pep.tile([P, CT * TQ], BF16, tag="fB", name="fB")            # 16KB
        gB = pep.tile([P, FFB * TQ], BF16, tag="gB", name="gB")           # 24KB
        for t2 in range(TQT):
            ss = peps.tile([P, NT], F32, tag="sse", name="sse")
            for ci in range(CT):
                sq = pep.tile([P, NT], BF16, tag="sqe", bufs=2, name="sqe")
                nc.scalar.activation(sq[:], hB[:, ci * TQ + t2 * NT: ci * TQ + (t2 + 1) * NT], AF.Square)
                nc.tensor.matmul(ss[:], ones_t[:], sq[:], start=(ci == 0), stop=(ci == CT - 1))
            sqt = pep.tile([P, NT], F32, tag="sqte", bufs=2, name="sqte")
            nc.scalar.activation(sqt[:], ss[:], AF.Sqrt, scale=1.0 / C, bias=eps_t[:])
            rn = pep.tile([P, NT], F32, tag="rne", bufs=2, name="rne")
            nc.vector.reciprocal(rn[:], sqt[:])
            for ci in range(CT):
                nc.vector.tensor_mul(fB[:, ci * TQ + t2 * NT: ci * TQ + (t2 + 1) * NT],
                                     hB[:, ci * TQ + t2 * NT: ci * TQ + (t2 + 1) * NT], rn[:])
        for fb in range(FFB):
            for t2 in range(TQT):
                pu = peps.tile([P, NT], F32, tag="pu", name="pu")
                for ci in range(CT):
                    nc.tensor.matmul(pu[:],
                                     w1_sb[:, ci * FF + fb * P: ci * FF + (fb + 1) * P],
                                     fB[:, ci * TQ + t2 * NT: ci * TQ + (t2 + 1) * NT],
                                     start=(ci == 0), stop=(ci == CT - 1))
                nc.scalar.activation(gB[:, fb * TQ + t2 * NT: fb * TQ + (t2 + 1) * NT],
                                     pu[:], AF.Gelu)
        for cb in range(CT):
            for t2 in range(TQT):
                py = peps.tile([P, NT], F32, tag="py", name="py")
                for fb in range(FFB):
                    nc.tensor.matmul(py[:],
                                     w2_sb[:, fb * C + cb * P: fb * C + (cb + 1) * P],
                                     gB[:, fb * TQ + t2 * NT: fb * TQ + (t2 + 1) * NT],
                                     start=(fb == 0), stop=(fb == FFB - 1))
                yt = pep.tile([P, NT], F32, tag="yt", bufs=3, name="yt")
                nc.vector.tensor_add(yt[:], py[:], hB[:, cb * TQ + t2 * NT: cb * TQ + (t2 + 1) * NT])
                nc.sync.dma_start(out[cb * P:(cb + 1) * P, t2 * NT:(t2 + 1) * NT], yt[:])
        peps_cm.__exit__(None, None, None)
        pe_cm.__exit__(None, None, None)
        h_cm.__exit__(None, None, None)
        cpool_cm.__exit__(None, None, None)

    nc.finalize()
    return nc


def get_nc():
    if "nc" not in _CACHE:
        _CACHE["nc"] = _build()
    return _CACHE["nc"]


def _prep_inputs(inputs):
    bf = ml_dtypes.bfloat16
    x = np.asarray(inputs["x"], dtype=np.float32)
    g_attn = np.asarray(inputs["g_attn"], dtype=np.float32)
    g_ff = np.asarray(inputs["g_ff"], dtype=np.float32)
    wqb = (g_attn[:, None] * np.asarray(inputs["Wq"], np.float32)).astype(bf)
    wkb = (g_attn[:, None] * np.asarray(inputs["Wk"], np.float32)).astype(bf)
    wvb = (g_attn[:, None] * np.asarray(inputs["Wv"], np.float32)).astype(bf)
    wob = np.asarray(inputs["Wo"], np.float32).astype(bf)
    w1b = (g_ff[:, None] * np.asarray(inputs["W1"], np.float32)).astype(bf)
    w2b = np.asarray(inputs["W2"], np.float32).astype(bf)
    in_maps = []
    for core in range(8):
        b, cq = divmod(core, 4)
        in_maps.append({
            "xb": x[b],
            "xq": np.ascontiguousarray(x[b][:, cq * TQ:(cq + 1) * TQ]),
            "wq": wqb, "wk": wkb, "wv": wvb, "wo": wob, "w1": w1b, "w2": w2b,
        })
    return in_maps


def run(inputs, **kwargs):
    nc = get_nc()
    in_maps = _prep_inputs(inputs)
    res = run_bass_kernel_spmd(nc, in_maps, core_ids=list(range(8)), **kwargs)
    out = np.empty((B, C, T), np.float32)
    for core in range(8):
        b, cq = divmod(core, 4)
        out[b][:, cq * TQ:(cq + 1) * TQ] = res.results[core]["out"]
    return out, res


def kernel(**inputs) -> np.ndarray:
    out, _ = run(inputs)
    return out
